# revision 10
# baseline (speedup 1.0000x reference)
"""Cross-attention block kernel for 8 TRN2 NeuronCores.

Math (per batch element b, one per core):
    Q = q @ Wq^T            [Lq, 128]
    K = k @ Wk^T            [Lkv, 128]
    V = v @ Wv^T            [Lkv, 128]
    S = Q @ K^T * d^-0.5    [Lq, Lkv]
    O = softmax(S) @ V      [Lq, 128]

Device strategy (per core):
  - HWDGE DMA loads q/k/v fp32 from HBM in 512-row slabs, natural layout.
  - PE transposes (matmul against identity) produce feat-major bf16 tiles;
    the PSUM->SBUF evacuation does the fp32->bf16 cast and is split between
    the vector and scalar engines.
  - Projections: QT/KT computed as [d, seq] (weights stationary); V computed
    in natural [seq, d] layout (v^T tiles stationary, Wv^T moving).
  - S^T tiles [k,q] = KT_slice.T @ QT; softmax runs WITHOUT max subtraction
    (scores are ~N(0,1); exp is safe in fp32) so exp+scale is a single
    scalar-engine activation pass straight out of PSUM.
  - P^T tiles feed PV matmuls as the stationary operand against an
    augmented moving operand [V | 1]: the extra ones column makes the PSUM
    accumulator [q,129] hold both O_unnorm and the softmax denominator.
  - Normalization is a per-partition reciprocal + tensor_scalar multiply.
"""

import os
import numpy as np
import ml_dtypes

from contextlib import ExitStack

import concourse.bass as bass
import concourse.tile as tile
from concourse import bacc, mybir
from concourse.bass_utils import run_bass_kernel_spmd

F32 = mybir.dt.float32
BF16 = mybir.dt.bfloat16

B = 8
LQ = 2048
LKV = 2048
DQ = 512
DKV = 768
D = 128
N_CORES = 8

_cache = {}


def build_program(Lq=LQ, Lkv=LKV, Dq=DQ, Dkv=DKV):
    assert Lq % 128 == 0 and Lkv % 128 == 0 and Dq % 128 == 0 and Dkv % 128 == 0
    nc = bacc.Bacc("TRN2", target_bir_lowering=False)

    q_d = nc.declare_dram_parameter("q", [Lq, Dq], F32, isOutput=False)
    k_d = nc.declare_dram_parameter("k", [Lkv, Dkv], F32, isOutput=False)
    v_d = nc.declare_dram_parameter("v", [Lkv, Dkv], F32, isOutput=False)
    wq_d = nc.declare_dram_parameter("wqT", [Dq, D], BF16, isOutput=False)
    wk_d = nc.declare_dram_parameter("wkT", [Dkv, D], BF16, isOutput=False)
    wv_d = nc.declare_dram_parameter("wvT", [Dkv, D], BF16, isOutput=False)
    out_d = nc.declare_dram_parameter("out", [Lq, D], F32, isOutput=True)

    with tile.TileContext(nc) as tc:
        _body(tc, q_d, k_d, v_d, wq_d, wk_d, wv_d, out_d, Lq, Lkv, Dq, Dkv)
    nc.compile()
    return nc


def _body(tc, q_d, k_d, v_d, wq_d, wk_d, wv_d, out_d, Lq, Lkv, Dq, Dkv):
    nc = tc.nc
    scale = float(D) ** -0.5
    F32R = mybir.dt.float32r
    ICQ = Dq // 128   # q feature chunks
    ICK = Dkv // 128  # k/v feature chunks
    NKT = Lkv // 128  # kv seq tiles
    QCW = 512 if Lq % 512 == 0 else 128   # q chunk width for attention
    NQC = Lq // QCW
    QSUB = QCW // 128
    NQT = Lq // 128
    SLABQ = 512 if Lq % 512 == 0 else 128
    SLABK = 512 if Lkv % 512 == 0 else 128
    KTPB = SLABK // 128  # kv tiles per block

    with ExitStack() as ctx:
        # -------- SBUF pools --------
        wpool = ctx.enter_context(tc.tile_pool(name="weights", bufs=1))
        xtp = ctx.enter_context(tc.tile_pool(name="xT", bufs=1))
        projp = ctx.enter_context(tc.tile_pool(name="proj", bufs=1))
        stag = ctx.enter_context(tc.tile_pool(name="stage", bufs=3))
        ptp = ctx.enter_context(tc.tile_pool(name="probs", bufs=6))
        outp = ctx.enter_context(tc.tile_pool(name="outs", bufs=4))

        # -------- PSUM pools (8 banks: ptr 2 + ps 2 + pss 1 + pvacc 3) ----
        ptrp = ctx.enter_context(tc.tile_pool(name="psum_tr", bufs=2, space="PSUM"))
        psp = ctx.enter_context(tc.tile_pool(name="psum_proj", bufs=2, space="PSUM"))
        pssp = ctx.enter_context(tc.tile_pool(name="psum_s", bufs=1, space="PSUM"))
        pvap = ctx.enter_context(tc.tile_pool(name="psum_pv", bufs=3, space="PSUM"))

        # weights, already transposed on host: [Din, D] -> sbuf [128, IC, D]
        wq_sb = wpool.tile([128, ICQ, D], BF16, name="wq_sb")
        wk_sb = wpool.tile([128, ICK, D], BF16, name="wk_sb")
        wv_sb = wpool.tile([128, ICK, D], BF16, name="wv_sb")
        nc.sync.dma_start(out=wq_sb[:], in_=wq_d[:].rearrange("(c p) d -> p c d", p=128))
        nc.sync.dma_start(out=wk_sb[:], in_=wk_d[:].rearrange("(c p) d -> p c d", p=128))
        nc.sync.dma_start(out=wv_sb[:], in_=wv_d[:].rearrange("(c p) d -> p c d", p=128))

        # transposed inputs, bf16: xT[p=feat128, chunk, seq]
        qT = xtp.tile([128, ICQ, Lq], BF16, name="qT")
        kT = xtp.tile([128, ICK, Lkv], BF16, name="kT")
        vT = xtp.tile([128, ICK, Lkv], BF16, name="vT")

        # projections
        QT = projp.tile([128, Lq], BF16, name="QT")     # [d, q]
        KT = projp.tile([128, Lkv], BF16, name="KT")    # [d, k]
        Vn = projp.tile([128, NKT, D + 1], BF16, name="Vn")  # natural V + ones

        # O accumulator in SBUF: [128, q_tile, D+1] fp32
        O_acc = projp.tile([128, NQT, D + 1], F32, name="O_acc")
        nc.gpsimd.memset(O_acc[:], 0.0)

        # identity for PE transposes (fp32 build; bitcast to fp32r at use)
        ident = wpool.tile([128, 128], F32, name="ident")
        from concourse.masks import make_identity
        make_identity(nc, ident[:])

        # ones column for the fused denominator trick (data cols overwritten)
        nc.vector.memset(Vn[:], 1.0)

        evac_flip = [0]

        def load_transpose(x_d, xT_sb, slab, ic_n, tag, sl):
            """Load one fp32 slab, PE-transpose its 128x128 tiles into PSUM
            as float32r (same bits, 25% faster through the PE), evacuate each
            bank to bf16."""
            nt = slab // 128
            nat = stag.tile([128, nt, ic_n * 128], F32,
                            name=f"nat_{tag}_{sl}", tag="nat")
            nc.sync.dma_start(
                out=nat[:],
                in_=x_d[:][sl * slab:(sl + 1) * slab, :]
                .rearrange("(t p) i -> p t i", p=128))
            for ic in range(ic_n):
                ptr = ptrp.tile([128, slab], F32R,
                                name=f"ptr_{tag}_{sl}_{ic}", tag="ptr")
                for t in range(nt):
                    nc.tensor.transpose(
                        ptr[:, t * 128:(t + 1) * 128],
                        nat[:, t, ic * 128:(ic + 1) * 128].bitcast(F32R),
                        ident[:].bitcast(F32R))
                dst = xT_sb[:, ic, sl * slab:(sl + 1) * slab]
                # evac with cast; DVE gets 2 of 3 (ACT is busy with exp)
                if evac_flip[0] % 3 == 2:
                    nc.scalar.copy(dst, ptr[:].bitcast(F32))
                else:
                    nc.vector.tensor_copy(dst, ptr[:].bitcast(F32))
                evac_flip[0] += 1

        def project_chunk(w_sb, x_sb, out_sb, ic_n, sc, ncols, tag):
            ps = psp.tile([128, ncols], F32, name=f"ps_{tag}_{sc}", tag="ps")
            for ic in range(ic_n):
                nc.tensor.matmul(
                    ps[:],
                    lhsT=w_sb[:, ic, :],
                    rhs=x_sb[:, ic, sc * ncols:(sc + 1) * ncols],
                    start=(ic == 0), stop=(ic == ic_n - 1))
            nc.vector.tensor_copy(out_sb[:, sc * ncols:(sc + 1) * ncols], ps[:])

        def project_v(kt):
            psv = psp.tile([128, D], F32, name=f"psv_{kt}", tag="ps")
            for ic in range(ICK):
                nc.tensor.matmul(
                    psv[:],
                    lhsT=vT[:, ic, kt * 128:(kt + 1) * 128],
                    rhs=wv_sb[:, ic, :],
                    start=(ic == 0), stop=(ic == ICK - 1))
            nc.vector.tensor_copy(Vn[:, kt, 0:D], psv[:])

        # -------- q first: attention needs full QT --------
        for sl in range(Lq // SLABQ):
            load_transpose(q_d, qT, SLABQ, ICQ, "q", sl)
            ncols = min(512, SLABQ)
            for j in range(SLABQ // ncols):
                sc = sl * (SLABQ // ncols) + j
                project_chunk(wq_sb, qT, QT, ICQ, sc, ncols, "q")

        # -------- k/v blocks, each followed by its attention slice --------
        for bj in range(Lkv // SLABK):
            load_transpose(k_d, kT, SLABK, ICK, "k", bj)
            load_transpose(v_d, vT, SLABK, ICK, "v", bj)
            ncols = min(512, SLABK)
            for j in range(SLABK // ncols):
                sc = bj * (SLABK // ncols) + j
                project_chunk(wk_sb, kT, KT, ICK, sc, ncols, "k")
            kts = range(bj * KTPB, (bj + 1) * KTPB)
            for kt in kts:
                project_v(kt)

            # attention for this kv block, all q chunks
            for qc in range(NQC):
                pts = []
                for kt in kts:
                    pss = pssp.tile([128, QCW], F32,
                                    name=f"pss_{bj}_{qc}_{kt}", tag="pss")
                    nc.tensor.matmul(
                        pss[:],
                        lhsT=KT[:, kt * 128:(kt + 1) * 128],
                        rhs=QT[:, qc * QCW:(qc + 1) * QCW],
                        start=True, stop=True)
                    pt = ptp.tile([128, QCW], BF16,
                                  name=f"pt_{bj}_{qc}_{kt}", tag="pt")
                    nc.scalar.activation(
                        pt[:], pss[:], mybir.ActivationFunctionType.Exp,
                        scale=scale)
                    pts.append(pt)
                for qs in range(QSUB):
                    pv = pvap.tile([128, D + 1], F32,
                                   name=f"pv_{bj}_{qc}_{qs}", tag="pv")
                    for i, kt in enumerate(kts):
                        nc.tensor.matmul(
                            pv[:],
                            lhsT=pts[i][:, qs * 128:(qs + 1) * 128],
                            rhs=Vn[:, kt, :],
                            start=(i == 0), stop=(i == len(pts) - 1))
                    t = qc * QSUB + qs
                    nc.vector.tensor_add(O_acc[:, t, :], O_acc[:, t, :], pv[:])

        # -------- normalize + store --------
        for t in range(NQT):
            r = outp.tile([128, 1], F32, name=f"r_{t}", tag="r")
            nc.vector.reciprocal(r[:], O_acc[:, t, D:D + 1])
            o = outp.tile([128, D], F32, name=f"o_{t}", tag="o")
            nc.vector.tensor_scalar_mul(o[:], O_acc[:, t, 0:D], r[:])
            nc.sync.dma_start(out=out_d[:][t * 128:(t + 1) * 128, :], in_=o[:])


def _get_program():
    key = (LQ, LKV, DQ, DKV)
    if key not in _cache:
        _cache[key] = build_program(*key)
    return _cache[key]


def kernel(q_input, k_input, v_input, Wq, Wk, Wv):
    out_dtype = q_input.dtype
    nc = _get_program()

    wqT = np.ascontiguousarray(Wq.T).astype(ml_dtypes.bfloat16)
    wkT = np.ascontiguousarray(Wk.T).astype(ml_dtypes.bfloat16)
    wvT = np.ascontiguousarray(Wv.T).astype(ml_dtypes.bfloat16)

    in_maps = []
    for c in range(N_CORES):
        in_maps.append({
            "q": np.ascontiguousarray(q_input[c]).astype(np.float32),
            "k": np.ascontiguousarray(k_input[c]).astype(np.float32),
            "v": np.ascontiguousarray(v_input[c]).astype(np.float32),
            "wqT": wqT, "wkT": wkT, "wvT": wvT,
        })

    trace = bool(int(os.environ.get("KERNEL_TRACE", "0")))
    res = run_bass_kernel_spmd(nc, in_maps, list(range(N_CORES)), trace=trace)
    kernel.last_results = res

    out = np.stack([res.results[c]["out"] for c in range(N_CORES)], axis=0)
    return out.astype(out_dtype)


# revision 12
# speedup vs baseline: 1.3747x; 1.3747x over previous
"""Cross-attention block kernel for 8 TRN2 NeuronCores.

Math (per batch element b, one per core):
    Q = q @ Wq^T            [Lq, 128]
    K = k @ Wk^T            [Lkv, 128]
    V = v @ Wv^T            [Lkv, 128]
    S = Q @ K^T * d^-0.5    [Lq, Lkv]
    O = softmax(S) @ V      [Lq, 128]

Device strategy (per core):
  - HWDGE DMA loads q/k/v fp32 from HBM in 512-row slabs, natural layout.
  - PE transposes (matmul against identity) produce feat-major bf16 tiles;
    the PSUM->SBUF evacuation does the fp32->bf16 cast and is split between
    the vector and scalar engines.
  - Projections: QT/KT computed as [d, seq] (weights stationary); V computed
    in natural [seq, d] layout (v^T tiles stationary, Wv^T moving).
  - S^T tiles [k,q] = KT_slice.T @ QT; softmax runs WITHOUT max subtraction
    (scores are ~N(0,1); exp is safe in fp32) so exp+scale is a single
    scalar-engine activation pass straight out of PSUM.
  - P^T tiles feed PV matmuls as the stationary operand against an
    augmented moving operand [V | 1]: the extra ones column makes the PSUM
    accumulator [q,129] hold both O_unnorm and the softmax denominator.
  - Normalization is a per-partition reciprocal + tensor_scalar multiply.
"""

import os
import numpy as np
import ml_dtypes

from contextlib import ExitStack

import concourse.bass as bass
import concourse.tile as tile
from concourse import bacc, mybir
from concourse.bass_utils import run_bass_kernel_spmd

F32 = mybir.dt.float32
BF16 = mybir.dt.bfloat16

B = 8
LQ = 2048
LKV = 2048
DQ = 512
DKV = 768
D = 128
N_CORES = 8

_cache = {}


def build_program(Lq=LQ, Lkv=LKV, Dq=DQ, Dkv=DKV):
    assert Lq % 128 == 0 and Lkv % 128 == 0 and Dq % 128 == 0 and Dkv % 128 == 0
    nc = bacc.Bacc("TRN2", target_bir_lowering=False)

    q_d = nc.declare_dram_parameter("q", [Lq, Dq], F32, isOutput=False)
    k_d = nc.declare_dram_parameter("k", [Lkv, Dkv], F32, isOutput=False)
    v_d = nc.declare_dram_parameter("v", [Lkv, Dkv], F32, isOutput=False)
    wq_d = nc.declare_dram_parameter("wqT", [Dq, D], BF16, isOutput=False)
    wk_d = nc.declare_dram_parameter("wkT", [Dkv, D], BF16, isOutput=False)
    wv_d = nc.declare_dram_parameter("wvT", [Dkv, D], BF16, isOutput=False)
    out_d = nc.declare_dram_parameter("out", [Lq, D], F32, isOutput=True)

    with tile.TileContext(nc) as tc:
        _body(tc, q_d, k_d, v_d, wq_d, wk_d, wv_d, out_d, Lq, Lkv, Dq, Dkv)
    nc.compile()
    return nc


def _body(tc, q_d, k_d, v_d, wq_d, wk_d, wv_d, out_d, Lq, Lkv, Dq, Dkv):
    nc = tc.nc
    scale = float(D) ** -0.5
    F32R = mybir.dt.float32r
    ICQ = Dq // 128   # q feature chunks
    ICK = Dkv // 128  # k/v feature chunks
    NKT = Lkv // 128  # kv seq tiles
    QCW = 512 if Lq % 512 == 0 else 128   # q chunk width for attention
    NQC = Lq // QCW
    QSUB = QCW // 128
    NQT = Lq // 128
    SLABQ = 512 if Lq % 512 == 0 else 128
    SLABK = 512 if Lkv % 512 == 0 else 128
    KTPB = SLABK // 128  # kv tiles per block

    with ExitStack() as ctx:
        # -------- SBUF pools --------
        wpool = ctx.enter_context(tc.tile_pool(name="weights", bufs=1))
        xtp = ctx.enter_context(tc.tile_pool(name="xT", bufs=1))
        projp = ctx.enter_context(tc.tile_pool(name="proj", bufs=1))
        stag = ctx.enter_context(tc.tile_pool(name="stage", bufs=3))
        ptp = ctx.enter_context(tc.tile_pool(name="probs", bufs=6))
        outp = ctx.enter_context(tc.tile_pool(name="outs", bufs=4))

        # -------- PSUM pools (8 banks: ptr 2 + ps 2 + pss 1 + pvacc 3) ----
        ptrp = ctx.enter_context(tc.tile_pool(name="psum_tr", bufs=2, space="PSUM"))
        psp = ctx.enter_context(tc.tile_pool(name="psum_proj", bufs=2, space="PSUM"))
        pssp = ctx.enter_context(tc.tile_pool(name="psum_s", bufs=1, space="PSUM"))
        pvap = ctx.enter_context(tc.tile_pool(name="psum_pv", bufs=3, space="PSUM"))

        # weights, already transposed on host: [Din, D] -> sbuf [128, IC, D]
        wq_sb = wpool.tile([128, ICQ, D], BF16, name="wq_sb")
        wk_sb = wpool.tile([128, ICK, D], BF16, name="wk_sb")
        wv_sb = wpool.tile([128, ICK, D], BF16, name="wv_sb")
        nc.sync.dma_start(out=wq_sb[:], in_=wq_d[:].rearrange("(c p) d -> p c d", p=128))
        nc.sync.dma_start(out=wk_sb[:], in_=wk_d[:].rearrange("(c p) d -> p c d", p=128))
        nc.sync.dma_start(out=wv_sb[:], in_=wv_d[:].rearrange("(c p) d -> p c d", p=128))

        # transposed inputs, bf16: xT[p=feat128, chunk, seq]
        qT = xtp.tile([128, ICQ, Lq], BF16, name="qT")
        kT = xtp.tile([128, ICK, Lkv], BF16, name="kT")
        vT = xtp.tile([128, ICK, Lkv], BF16, name="vT")

        # projections
        QT = projp.tile([128, Lq], BF16, name="QT")     # [d, q]
        KT = projp.tile([128, Lkv], BF16, name="KT")    # [d, k]
        Vn = projp.tile([128, NKT, D + 1], BF16, name="Vn")  # natural V + ones

        # O accumulator in SBUF: [128, q_tile, D+1] fp32
        O_acc = projp.tile([128, NQT, D + 1], F32, name="O_acc")
        nc.gpsimd.memset(O_acc[:], 0.0)

        # identity for PE transposes (fp32 build; bitcast to fp32r at use)
        ident = wpool.tile([128, 128], F32, name="ident")
        from concourse.masks import make_identity
        make_identity(nc, ident[:])

        # ones column for the fused denominator trick (data cols overwritten)
        nc.vector.memset(Vn[:], 1.0)

        evac_flip = [0]

        def load_transpose(x_d, xT_sb, slab, ic_n, tag, sl):
            """Load one fp32 slab, PE-transpose its 128x128 tiles into PSUM
            (fp32), evacuate each bank to bf16 via DVE/ACT copies."""
            nt = slab // 128
            nat = stag.tile([128, nt, ic_n * 128], F32,
                            name=f"nat_{tag}_{sl}", tag="nat")
            nc.sync.dma_start(
                out=nat[:],
                in_=x_d[:][sl * slab:(sl + 1) * slab, :]
                .rearrange("(t p) i -> p t i", p=128))
            for ic in range(ic_n):
                ptr = ptrp.tile([128, slab], F32,
                                name=f"ptr_{tag}_{sl}_{ic}", tag="ptr")
                for t in range(nt):
                    nc.tensor.transpose(
                        ptr[:, t * 128:(t + 1) * 128],
                        nat[:, t, ic * 128:(ic + 1) * 128],
                        ident[:])
                dst = xT_sb[:, ic, sl * slab:(sl + 1) * slab]
                # evac with cast; DVE gets 2 of 3 (ACT is busy with exp)
                if evac_flip[0] % 3 == 2:
                    nc.scalar.copy(dst, ptr[:])
                else:
                    nc.vector.tensor_copy(dst, ptr[:])
                evac_flip[0] += 1

        def project_chunk(w_sb, x_sb, out_sb, ic_n, sc, ncols, tag):
            ps = psp.tile([128, ncols], F32, name=f"ps_{tag}_{sc}", tag="ps")
            for ic in range(ic_n):
                nc.tensor.matmul(
                    ps[:],
                    lhsT=w_sb[:, ic, :],
                    rhs=x_sb[:, ic, sc * ncols:(sc + 1) * ncols],
                    start=(ic == 0), stop=(ic == ic_n - 1))
            nc.vector.tensor_copy(out_sb[:, sc * ncols:(sc + 1) * ncols], ps[:])

        def project_v(kt):
            psv = psp.tile([128, D], F32, name=f"psv_{kt}", tag="ps")
            for ic in range(ICK):
                nc.tensor.matmul(
                    psv[:],
                    lhsT=vT[:, ic, kt * 128:(kt + 1) * 128],
                    rhs=wv_sb[:, ic, :],
                    start=(ic == 0), stop=(ic == ICK - 1))
            nc.vector.tensor_copy(Vn[:, kt, 0:D], psv[:])

        # -------- q first: attention needs full QT --------
        for sl in range(Lq // SLABQ):
            load_transpose(q_d, qT, SLABQ, ICQ, "q", sl)
            ncols = min(512, SLABQ)
            for j in range(SLABQ // ncols):
                sc = sl * (SLABQ // ncols) + j
                project_chunk(wq_sb, qT, QT, ICQ, sc, ncols, "q")

        # -------- k/v blocks, each followed by its attention slice --------
        for bj in range(Lkv // SLABK):
            load_transpose(k_d, kT, SLABK, ICK, "k", bj)
            load_transpose(v_d, vT, SLABK, ICK, "v", bj)
            ncols = min(512, SLABK)
            for j in range(SLABK // ncols):
                sc = bj * (SLABK // ncols) + j
                project_chunk(wk_sb, kT, KT, ICK, sc, ncols, "k")
            kts = range(bj * KTPB, (bj + 1) * KTPB)
            for kt in kts:
                project_v(kt)

            # attention for this kv block, all q chunks
            for qc in range(NQC):
                pts = []
                for kt in kts:
                    pss = pssp.tile([128, QCW], F32,
                                    name=f"pss_{bj}_{qc}_{kt}", tag="pss")
                    nc.tensor.matmul(
                        pss[:],
                        lhsT=KT[:, kt * 128:(kt + 1) * 128],
                        rhs=QT[:, qc * QCW:(qc + 1) * QCW],
                        start=True, stop=True)
                    pt = ptp.tile([128, QCW], BF16,
                                  name=f"pt_{bj}_{qc}_{kt}", tag="pt")
                    nc.scalar.activation(
                        pt[:], pss[:], mybir.ActivationFunctionType.Exp,
                        scale=scale)
                    pts.append(pt)
                for qs in range(QSUB):
                    pv = pvap.tile([128, D + 1], F32,
                                   name=f"pv_{bj}_{qc}_{qs}", tag="pv")
                    for i, kt in enumerate(kts):
                        nc.tensor.matmul(
                            pv[:],
                            lhsT=pts[i][:, qs * 128:(qs + 1) * 128],
                            rhs=Vn[:, kt, :],
                            start=(i == 0), stop=(i == len(pts) - 1))
                    t = qc * QSUB + qs
                    nc.vector.tensor_add(O_acc[:, t, :], O_acc[:, t, :], pv[:])

        # -------- normalize + store --------
        for t in range(NQT):
            r = outp.tile([128, 1], F32, name=f"r_{t}", tag="r")
            nc.vector.reciprocal(r[:], O_acc[:, t, D:D + 1])
            o = outp.tile([128, D], F32, name=f"o_{t}", tag="o")
            nc.vector.tensor_scalar_mul(o[:], O_acc[:, t, 0:D], r[:])
            nc.sync.dma_start(out=out_d[:][t * 128:(t + 1) * 128, :], in_=o[:])


def _get_program():
    key = (LQ, LKV, DQ, DKV)
    if key not in _cache:
        _cache[key] = build_program(*key)
    return _cache[key]


def kernel(q_input, k_input, v_input, Wq, Wk, Wv):
    out_dtype = q_input.dtype
    nc = _get_program()

    wqT = np.ascontiguousarray(Wq.T).astype(ml_dtypes.bfloat16)
    wkT = np.ascontiguousarray(Wk.T).astype(ml_dtypes.bfloat16)
    wvT = np.ascontiguousarray(Wv.T).astype(ml_dtypes.bfloat16)

    in_maps = []
    for c in range(N_CORES):
        in_maps.append({
            "q": np.ascontiguousarray(q_input[c]).astype(np.float32),
            "k": np.ascontiguousarray(k_input[c]).astype(np.float32),
            "v": np.ascontiguousarray(v_input[c]).astype(np.float32),
            "wqT": wqT, "wkT": wkT, "wvT": wvT,
        })

    trace = bool(int(os.environ.get("KERNEL_TRACE", "0")))
    if trace:
        try:
            import antenv.axon_hooks  # noqa: F401  (needed by the trace path)
        except ImportError:
            trace = False
    res = run_bass_kernel_spmd(nc, in_maps, list(range(N_CORES)), trace=trace)
    kernel.last_results = res

    out = np.stack([res.results[c]["out"] for c in range(N_CORES)], axis=0)
    return out.astype(out_dtype)
